# revision 2
# baseline (speedup 1.0000x reference)
"""Data-parallel linear layer (x @ W.T + bias) on 8 TRN2 NeuronCores.

Shard x over batch: each core computes a (1024 x 2048) @ (2048 x 2048).T
matmul in float32r (full-rate fp32 PE mode), bias added on DVE.
"""
import numpy as np

import concourse.bass as bass  # noqa: F401  (bass types referenced via tile/bacc)
import concourse.mybir as mybir
import concourse.tile as tile
from concourse import bacc, bass_utils

B, IN, OUT = 8192, 2048, 2048
NCORES = 8
BS = B // NCORES      # 1024 batch rows per core
P = 128               # partition dim
NFREE = 512           # fp32 moving-operand max / one PSUM bank
KT = IN // P          # 16 contraction tiles
MT = BS // P          # 8 output-row tiles per core
NT = OUT // NFREE     # 4 output-col tiles

F32 = mybir.dt.float32
F32R = mybir.dt.float32r

# set by test harness to enable NTFF profiling
TRACE = False
LAST_EXEC_NS = None

_NC_CACHE = {}


def _build():
    if "nc" in _NC_CACHE:
        return _NC_CACHE["nc"]
    nc = bacc.Bacc("TRN2", target_bir_lowering=False, debug=False)
    xT = nc.dram_tensor("xT", [IN, BS], F32R, kind="ExternalInput")
    wT = nc.dram_tensor("wT", [IN, OUT], F32R, kind="ExternalInput")
    bias_b = nc.dram_tensor("bias_b", [P, OUT], F32, kind="ExternalInput")
    out = nc.dram_tensor("out", [BS, OUT], F32, kind="ExternalOutput")

    xT_ap = xT.ap()
    wT_ap = wT.ap()
    out_ap = out.ap()

    with tile.TileContext(nc) as tc:
        with tc.tile_pool(name="xp", bufs=KT) as xp, \
             tc.tile_pool(name="wp", bufs=12) as wp, \
             tc.tile_pool(name="bp", bufs=1) as bp, \
             tc.tile_pool(name="op", bufs=8) as op, \
             tc.tile_pool(name="pp", bufs=8, space="PSUM") as pp:
            bias_sb = bp.tile([P, OUT], F32)
            nc.sync.dma_start(bias_sb[:], bias_b.ap())

            x_sb = [None] * KT
            for n in range(NT):
                w_sb = []
                for k in range(KT):
                    if n == 0:
                        t = xp.tile([P, BS], F32R, tag="x")
                        nc.sync.dma_start(t[:], xT_ap[k * P:(k + 1) * P, :])
                        x_sb[k] = t
                    wt_ = wp.tile([P, NFREE], F32R, tag="w")
                    nc.sync.dma_start(
                        wt_[:], wT_ap[k * P:(k + 1) * P,
                                      n * NFREE:(n + 1) * NFREE])
                    w_sb.append(wt_)

                ps = [pp.tile([P, NFREE], F32, tag="ps", name=f"ps_{n}_{m}")
                      for m in range(MT)]
                for k in range(KT):
                    for m in range(MT):
                        nc.tensor.matmul(
                            ps[m][:],
                            x_sb[k][:, m * P:(m + 1) * P],
                            w_sb[k][:],
                            start=(k == 0),
                            stop=(k == KT - 1),
                        )
                for m in range(MT):
                    ot = op.tile([P, NFREE], F32, tag="o")
                    nc.vector.tensor_add(
                        ot[:], ps[m][:],
                        bias_sb[:, n * NFREE:(n + 1) * NFREE])
                    nc.sync.dma_start(
                        out_ap[m * P:(m + 1) * P,
                               n * NFREE:(n + 1) * NFREE], ot[:])
    nc.compile()
    _NC_CACHE["nc"] = nc
    return nc


def kernel(x: np.ndarray, weight: np.ndarray, bias: np.ndarray) -> np.ndarray:
    global LAST_EXEC_NS
    x = np.asarray(x, dtype=np.float32)
    weight = np.asarray(weight, dtype=np.float32)
    bias = np.asarray(bias, dtype=np.float32)

    xT = np.ascontiguousarray(x.T)            # [IN, B]
    wT = np.ascontiguousarray(weight.T)       # [IN, OUT]
    bias_b = np.ascontiguousarray(
        np.broadcast_to(bias[None, :], (P, OUT)), dtype=np.float32)

    in_maps = [
        {
            "xT": np.ascontiguousarray(xT[:, c * BS:(c + 1) * BS]),
            "wT": wT,
            "bias_b": bias_b,
        }
        for c in range(NCORES)
    ]

    nc = _build()
    res = bass_utils.run_bass_kernel_spmd(
        nc, in_maps, core_ids=list(range(NCORES)), trace=TRACE)
    LAST_EXEC_NS = res.exec_time_ns

    return np.concatenate([r["out"] for r in res.results], axis=0)


# revision 3
# speedup vs baseline: 1.0549x; 1.0549x over previous
"""Data-parallel linear layer (x @ W.T + bias) on 8 TRN2 NeuronCores.

Shard x over batch: each core computes a (1024 x 2048) @ (2048 x 2048).T
matmul in float32r (full-rate fp32 PE mode), bias added on DVE.

Per-core schedule: 4 output-column blocks (n) of 512.
 - n=0,1: k-major (stream k-slabs of x and W; PSUM groups for all 8 m
   interleave per k) -- degrades gracefully while the 12 MiB n=0 input
   crunch is in flight.
 - n=2,3: m-major (16 k-contiguous matmuls per PSUM group) -- spreads
   the DVE bias-add drains and output DMAs evenly, so the kernel tail
   is one drain, not eight.
"""
import numpy as np

import concourse.bass as bass  # noqa: F401
import concourse.mybir as mybir
import concourse.tile as tile
from concourse import bacc, bass_utils

B, IN, OUT = 8192, 2048, 2048
NCORES = 8
BS = B // NCORES      # 1024 batch rows per core
P = 128               # partition dim
NFREE = 512           # fp32 moving-operand max / one PSUM bank
KT = IN // P          # 16 contraction tiles
MT = BS // P          # 8 output-row tiles per core
NT = OUT // NFREE     # 4 output-col tiles
XC = 512              # x DMA chunk (free dim)
XCN = BS // XC        # 2 chunks per x k-slab

F32 = mybir.dt.float32
F32R = mybir.dt.float32r

TRACE = False
LAST_EXEC_NS = None

_NC_CACHE = {}


def _build():
    if "nc" in _NC_CACHE:
        return _NC_CACHE["nc"]
    nc = bacc.Bacc("TRN2", target_bir_lowering=False, debug=False)
    xT = nc.dram_tensor("xT", [IN, BS], F32R, kind="ExternalInput")
    wT = nc.dram_tensor("wT", [IN, OUT], F32R, kind="ExternalInput")
    bias_b = nc.dram_tensor("bias_b", [P, OUT], F32, kind="ExternalInput")
    out = nc.dram_tensor("out", [BS, OUT], F32, kind="ExternalOutput")

    xT_ap = xT.ap()
    wT_ap = wT.ap()
    out_ap = out.ap()

    with tile.TileContext(nc) as tc:
        with tc.tile_pool(name="xp", bufs=KT * XCN) as xp, \
             tc.tile_pool(name="wp", bufs=24) as wp, \
             tc.tile_pool(name="bp", bufs=1) as bp, \
             tc.tile_pool(name="op", bufs=8) as op, \
             tc.tile_pool(name="pp", bufs=8, space="PSUM") as pp:
            bias_sb = bp.tile([P, OUT], F32)
            x_sb = [[None] * XCN for _ in range(KT)]
            w_sb = [[None] * KT for _ in range(NT)]

            def emit_x_dma(k):
                for c in range(XCN):
                    t = xp.tile([P, XC], F32R, tag="x", name=f"x_{k}_{c}")
                    nc.sync.dma_start(
                        t[:], xT_ap[k * P:(k + 1) * P, c * XC:(c + 1) * XC])
                    x_sb[k][c] = t

            def emit_w_dma(n, k):
                t = wp.tile([P, NFREE], F32R, tag="w", name=f"w_{n}_{k}")
                nc.sync.dma_start(
                    t[:], wT_ap[k * P:(k + 1) * P,
                                n * NFREE:(n + 1) * NFREE])
                w_sb[n][k] = t

            def mm(n, k, m, ps_m):
                xc = x_sb[k][m // (MT // XCN)]
                moff = (m % (MT // XCN)) * P
                nc.tensor.matmul(
                    ps_m[:],
                    xc[:, moff:moff + P],
                    w_sb[n][k][:],
                    start=(k == 0),
                    stop=(k == KT - 1),
                )

            def drain(n, m, ps_m):
                ot = op.tile([P, NFREE], F32, tag="o", name=f"o_{n}_{m}")
                nc.vector.tensor_add(
                    ot[:], ps_m[:], bias_sb[:, n * NFREE:(n + 1) * NFREE])
                nc.sync.dma_start(
                    out_ap[m * P:(m + 1) * P,
                           n * NFREE:(n + 1) * NFREE], ot[:])

            for n in range(NT):
                for k in range(KT):
                    if n == 0:
                        emit_x_dma(k)
                    emit_w_dma(n, k)
                    if n == 0 and k == KT // 2:
                        # bias needed only at the first drain; keep it
                        # out of the startup DMA crunch
                        nc.sync.dma_start(bias_sb[:], bias_b.ap())

                ps = [pp.tile([P, NFREE], F32, tag="ps", name=f"ps_{n}_{m}")
                      for m in range(MT)]
                if n < 2:
                    # k-major: all PSUM groups advance together per k-slab
                    for k in range(KT):
                        for m in range(MT):
                            mm(n, k, m, ps[m])
                    for m in range(MT):
                        drain(n, m, ps[m])
                else:
                    # m-major: k-contiguous chain per PSUM group,
                    # drain immediately after each chain
                    for m in range(MT):
                        for k in range(KT):
                            mm(n, k, m, ps[m])
                        drain(n, m, ps[m])
    nc.compile()
    _NC_CACHE["nc"] = nc
    return nc


def kernel(x: np.ndarray, weight: np.ndarray, bias: np.ndarray) -> np.ndarray:
    global LAST_EXEC_NS
    x = np.asarray(x, dtype=np.float32)
    weight = np.asarray(weight, dtype=np.float32)
    bias = np.asarray(bias, dtype=np.float32)

    xT = np.ascontiguousarray(x.T)            # [IN, B]
    wT = np.ascontiguousarray(weight.T)       # [IN, OUT]
    bias_b = np.ascontiguousarray(
        np.broadcast_to(bias[None, :], (P, OUT)), dtype=np.float32)

    in_maps = [
        {
            "xT": np.ascontiguousarray(xT[:, c * BS:(c + 1) * BS]),
            "wT": wT,
            "bias_b": bias_b,
        }
        for c in range(NCORES)
    ]

    nc = _build()
    res = bass_utils.run_bass_kernel_spmd(
        nc, in_maps, core_ids=list(range(NCORES)), trace=TRACE)
    LAST_EXEC_NS = res.exec_time_ns

    return np.concatenate([r["out"] for r in res.results], axis=0)


# revision 5
# speedup vs baseline: 1.0780x; 1.0219x over previous
"""Data-parallel linear layer (x @ W.T + bias) on 8 TRN2 NeuronCores.

Shard x over batch: each core computes a (1024 x 2048) @ (2048 x 2048).T
matmul in float32r (full-rate fp32 PE mode), bias added on DVE.

Per-core schedule: 4 output-column blocks (n) of 512.
 - n=0,1: k-major (stream k-slabs of x and W; PSUM groups for all 8 m
   interleave per k) -- degrades gracefully while the 12 MiB n=0 input
   crunch is in flight.
 - n=2,3: m-major (16 k-contiguous matmuls per PSUM group) -- spreads
   the DVE bias-add drains and output DMAs evenly, so the kernel tail
   is one drain, not eight.
"""
import numpy as np

import concourse.bass as bass  # noqa: F401
import concourse.mybir as mybir
import concourse.tile as tile
from concourse import bacc, bass_utils

B, IN, OUT = 8192, 2048, 2048
NCORES = 8
BS = B // NCORES      # 1024 batch rows per core
P = 128               # partition dim
NFREE = 512           # fp32 moving-operand max / one PSUM bank
KT = IN // P          # 16 contraction tiles
MT = BS // P          # 8 output-row tiles per core
NT = OUT // NFREE     # 4 output-col tiles
XC = 512              # x DMA chunk (free dim)
XCN = BS // XC        # 2 chunks per x k-slab

F32 = mybir.dt.float32
F32R = mybir.dt.float32r

TRACE = False
LAST_EXEC_NS = None

_NC_CACHE = {}


def _build():
    if "nc" in _NC_CACHE:
        return _NC_CACHE["nc"]
    nc = bacc.Bacc("TRN2", target_bir_lowering=False, debug=False)
    xT = nc.dram_tensor("xT", [IN, BS], F32R, kind="ExternalInput")
    wT = nc.dram_tensor("wT", [IN, OUT], F32R, kind="ExternalInput")
    bias_b = nc.dram_tensor("bias_b", [P, OUT], F32, kind="ExternalInput")
    out = nc.dram_tensor("out", [BS, OUT], F32, kind="ExternalOutput")

    xT_ap = xT.ap()
    wT_ap = wT.ap()
    out_ap = out.ap()

    with tile.TileContext(nc) as tc:
        with tc.tile_pool(name="xp", bufs=KT * XCN) as xp, \
             tc.tile_pool(name="wp", bufs=32) as wp, \
             tc.tile_pool(name="bp", bufs=1) as bp, \
             tc.tile_pool(name="op", bufs=8) as op, \
             tc.tile_pool(name="pp", bufs=8, space="PSUM") as pp:
            bias_sb = bp.tile([P, OUT], F32)
            x_sb = [[None] * XCN for _ in range(KT)]
            w_sb = [[None] * KT for _ in range(NT)]

            def emit_x_dma(k):
                for c in range(XCN):
                    t = xp.tile([P, XC], F32R, tag="x", name=f"x_{k}_{c}")
                    nc.sync.dma_start(
                        t[:], xT_ap[k * P:(k + 1) * P, c * XC:(c + 1) * XC])
                    x_sb[k][c] = t

            def emit_w_dma(n, k):
                t = wp.tile([P, NFREE], F32R, tag="w", name=f"w_{n}_{k}")
                nc.sync.dma_start(
                    t[:], wT_ap[k * P:(k + 1) * P,
                                n * NFREE:(n + 1) * NFREE])
                w_sb[n][k] = t

            def mm(n, k, m, ps_m):
                xc = x_sb[k][m // (MT // XCN)]
                moff = (m % (MT // XCN)) * P
                nc.tensor.matmul(
                    ps_m[:],
                    xc[:, moff:moff + P],
                    w_sb[n][k][:],
                    start=(k == 0),
                    stop=(k == KT - 1),
                )

            def drain(n, m, ps_m):
                ot = op.tile([P, NFREE], F32, tag="o", name=f"o_{n}_{m}")
                nc.vector.tensor_add(
                    ot[:], ps_m[:], bias_sb[:, n * NFREE:(n + 1) * NFREE])
                # SWDGE for outputs: keeps the HWDGE FIFOs input-only so
                # weight prefetch is never stuck behind output writes
                nc.gpsimd.dma_start(
                    out_ap[m * P:(m + 1) * P,
                           n * NFREE:(n + 1) * NFREE], ot[:])

            for n in range(NT):
                for k in range(KT):
                    if n == 0:
                        emit_x_dma(k)
                    emit_w_dma(n, k)
                    if n == 0 and k == KT // 2:
                        # bias needed only at the first drain; keep it
                        # out of the startup DMA crunch
                        nc.sync.dma_start(bias_sb[:], bias_b.ap())

                ps = [pp.tile([P, NFREE], F32, tag="ps", name=f"ps_{n}_{m}")
                      for m in range(MT)]
                if n < 2:
                    # k-major: all PSUM groups advance together per k-slab
                    for k in range(KT):
                        for m in range(MT):
                            mm(n, k, m, ps[m])
                    for m in range(MT):
                        drain(n, m, ps[m])
                else:
                    # m-major: k-contiguous chain per PSUM group,
                    # drain immediately after each chain
                    for m in range(MT):
                        for k in range(KT):
                            mm(n, k, m, ps[m])
                        drain(n, m, ps[m])
    nc.compile()
    _NC_CACHE["nc"] = nc
    return nc


def kernel(x: np.ndarray, weight: np.ndarray, bias: np.ndarray) -> np.ndarray:
    global LAST_EXEC_NS
    x = np.asarray(x, dtype=np.float32)
    weight = np.asarray(weight, dtype=np.float32)
    bias = np.asarray(bias, dtype=np.float32)

    xT = np.ascontiguousarray(x.T)            # [IN, B]
    wT = np.ascontiguousarray(weight.T)       # [IN, OUT]
    bias_b = np.ascontiguousarray(
        np.broadcast_to(bias[None, :], (P, OUT)), dtype=np.float32)

    in_maps = [
        {
            "xT": np.ascontiguousarray(xT[:, c * BS:(c + 1) * BS]),
            "wT": wT,
            "bias_b": bias_b,
        }
        for c in range(NCORES)
    ]

    nc = _build()
    res = bass_utils.run_bass_kernel_spmd(
        nc, in_maps, core_ids=list(range(NCORES)), trace=TRACE)
    LAST_EXEC_NS = res.exec_time_ns

    return np.concatenate([r["out"] for r in res.results], axis=0)
